# revision 37
# baseline (speedup 1.0000x reference)
"""Causal multi-head attention layer on 8 trn2 NeuronCores.

Sharding: 8 shards = 4 batches x 2 head-groups (8 heads each).
Each core computes, for its (batch b, head-group g):
  - Q/K projections transposed: qT/kT [512, 2048] (head-dim on partitions)
  - V projection in natural layout [2048, 512], stored interleaved with an
    all-ones block (even heads [v|ones], odd heads [ones|v]) so the AV
    matmul yields both sum(p*v) and sum(p) at once, with the two heads of a
    pair landing their denominators on opposite row halves
  - causal attention per head with scores computed transposed (S^T = K Q^T),
    exp on ScalarE (scale=1/8 folded in), no max subtraction (scores bounded
    for this input distribution)
  - softmax divide: both heads' denominators gathered into one [128,512]
    tile (2 DVE copies), one Ln+Exp pair on ScalarE, products on DVE
  - partial output projection (row-split): out_part = attn_g @ Wo[:, g].T
Host gathers: out[b] = out_part[b,g=0] + out_part[b,g=1] + bo.

Schedule: the attention score->exp->mask->AV chain is software-pipelined
with a 2-group lag (scores run ahead; AV matmuls trail by two groups), and
projection / output-projection tiles are emitted as PE filler inside the
ACT-paced attention stream so the tensor engine never idles.

All matmuls in bf16 (fp32 accumulation in PSUM).
"""
import os
from collections import deque
import numpy as np
import ml_dtypes

B, T, D = 4, 2048, 1024
NH, HD = 16, 64
G = 2                 # head groups
HPG = NH // G         # heads per core = 8
GD = D // G           # group dim = 512
P = 128
DC = D // P           # 8  d-chunks
OC = GD // P          # 4  output chunks per group
KC = T // P           # 16 k chunks
NQT = T // 512        # 4  q tiles
N_CORES = 8

BF16 = ml_dtypes.bfloat16

# ---------------------------------------------------------------------------
# BIR compatibility patch: the bundled walrus rejects two bass encodings.
# 1) EVENT_SEMAPHORE_RANGE_CLEAR raw-ISA -> rewrite to per-sem EventSemaphore
#    writes (sem-wr-imm 0, the legacy reset mechanism).
# 2) >1 sync-wait per instruction -> hoist extras onto preceding NoOps.
# ---------------------------------------------------------------------------
_patched = False


def _fix_bir(bir_json: bytes) -> bytes:
    import orjson
    bir = orjson.loads(bir_json)
    changed = False
    for fn in bir.get("functions", []):
        for blk in fn.get("blocks", []):
            new_insts = []
            for inst in blk.get("instructions", []):
                si = inst.get("sync_info") or {}
                waits = si.get("on_wait") or []
                if len(waits) > 1 and inst.get("opcode") != "ISA":
                    changed = True
                    for i, w in enumerate(waits[1:]):
                        new_insts.append({
                            "opcode": "NoOp",
                            "name": f"{inst['name']}_wait{i}",
                            "engine": inst.get("engine", "Pool"),
                            "debug": inst.get("debug"),
                            "ins": [], "outs": [],
                            "sync_info": {"on_update": [], "on_wait": [w]},
                        })
                    si = dict(si)
                    si["on_wait"] = waits[:1]
                    inst = dict(inst)
                    inst["sync_info"] = si
                if (inst.get("opcode") == "ISA"
                        and inst.get("op_name") == "EVENT_SEMAPHORE_RANGE_CLEAR"):
                    ad = inst.get("ant_dict", {})
                    changed = True
                    # round-robin the per-sem clears across all engines: they
                    # run before everything, so spreading keeps each engine's
                    # share ~3us instead of serializing on one queue
                    engines = ["Pool", "Activation", "DVE", "PE", "SP"]
                    orig_wait = (inst.get("sync_info") or {}).get("on_wait", [])
                    seen_eng = set()
                    for i, sem in enumerate(range(ad["range_first"],
                                                  ad["range_last"] + 1)):
                        eng = engines[i % len(engines)]
                        first_on_eng = eng not in seen_eng
                        seen_eng.add(eng)
                        new_insts.append({
                            "opcode": "EventSemaphore",
                            "name": f"{inst['name']}_wr{i}",
                            "engine": eng,
                            "debug": inst.get("debug"),
                            "ins": [], "outs": [],
                            "sync_info": {
                                "on_update": [{
                                    "sync_type": "semaphore", "id": sem,
                                    "update_mode": "sem-wr-imm",
                                    "update_value": 0,
                                    "ant_name": f"semclear_{sem}",
                                }],
                                "on_wait": orig_wait if first_on_eng else [],
                            },
                        })
                else:
                    new_insts.append(inst)
            blk["instructions"] = new_insts
    return orjson.dumps(bir) if changed else bir_json


def _patch_compile():
    global _patched
    if _patched:
        return
    _patched = True
    import concourse.bass_utils as bu
    import concourse.bass2jax as b2j
    orig = bu.compile_bir_kernel

    def wrapped(bir_json, tmpdir, neff_name="file.neff"):
        return orig(_fix_bir(bir_json), tmpdir, neff_name)

    bu.compile_bir_kernel = wrapped
    if hasattr(b2j, "compile_bir_kernel"):
        b2j.compile_bir_kernel = wrapped

    # no-egress sandbox: skip artifact upload in the trace path
    bu.upload_artifacts = lambda tmpdir: f"local:{tmpdir}"

    # provide antenv.axon_hooks (missing in this image) so trace=True can
    # reach the NTFF profiling hook in libaxon_pjrt.so
    import sys as _sys
    import types as _types
    if "antenv.axon_hooks" not in _sys.modules:
        try:
            import antenv
            mod = _types.ModuleType("antenv.axon_hooks")
            mod._hook = None
            mod.set_axon_ntff_profile_hook = lambda h: setattr(mod, "_hook", h)
            mod.get_axon_ntff_profile_hook = lambda: mod._hook
            _sys.modules["antenv.axon_hooks"] = mod
            antenv.axon_hooks = mod
            so_path = "/opt/axon/libaxon_pjrt.so"
            if os.path.exists(so_path):
                from trn_agent_boot.trn_boot import _ntff_profile_via_ctypes
                mod._hook = _ntff_profile_via_ctypes(so_path)
        except Exception:
            pass


# ---------------------------------------------------------------------------
# Bass program (identical on all 8 cores; data differs per core)
# ---------------------------------------------------------------------------
_nc_cache = None


def build_nc():
    global _nc_cache
    if _nc_cache is not None:
        return _nc_cache
    import concourse.bass as bass
    import concourse.mybir as mybir
    from concourse.tile import TileContext

    f32 = mybir.dt.float32
    bf16 = mybir.dt.bfloat16
    Exp = mybir.ActivationFunctionType.Exp
    Ln = mybir.ActivationFunctionType.Ln

    nc = bass.Bass()
    xqT = nc.dram_tensor("xqT", [D, T], bf16, kind="ExternalInput")
    xkT = nc.dram_tensor("xkT", [D, T], bf16, kind="ExternalInput")
    xvT = nc.dram_tensor("xvT", [D, T], bf16, kind="ExternalInput")
    wqT = nc.dram_tensor("wqT", [D, GD], bf16, kind="ExternalInput")
    wkT = nc.dram_tensor("wkT", [D, GD], bf16, kind="ExternalInput")
    wvT = nc.dram_tensor("wvT", [D, GD], bf16, kind="ExternalInput")
    woT = nc.dram_tensor("woT", [GD, D], bf16, kind="ExternalInput")
    bq2 = nc.dram_tensor("bq2", [P, OC], f32, kind="ExternalInput")
    bk2 = nc.dram_tensor("bk2", [P, OC], f32, kind="ExternalInput")
    bv2 = nc.dram_tensor("bv2", [P, OC], f32, kind="ExternalInput")
    out_p = nc.dram_tensor("out_p", [T, D], f32, kind="ExternalOutput")

    xqT3 = xqT.rearrange("(dc p) t -> p dc t", p=P)
    xkT3 = xkT.rearrange("(dc p) t -> p dc t", p=P)
    xvT3 = xvT.rearrange("(dc p) t -> p dc t", p=P)
    wq4 = wqT.rearrange("(dc p) o -> p dc o", p=P)
    wk4 = wkT.rearrange("(dc p) o -> p dc o", p=P)
    wv4 = wvT.rearrange("(dc p) o -> p dc o", p=P)
    wo4 = woT.rearrange("(cc p) o -> p cc o", p=P)

    with TileContext(nc) as tc:
        with tc.tile_pool(name="consts", bufs=1) as consts, \
             tc.tile_pool(name="wpool", bufs=1) as wpool, \
             tc.tile_pool(name="stage", bufs=4) as stage, \
             tc.tile_pool(name="expp", bufs=8) as expp, \
             tc.tile_pool(name="small", bufs=2) as small, \
             tc.tile_pool(name="outst", bufs=4) as outst, \
             tc.tile_pool(name="ps_proj", bufs=2, space="PSUM") as ps_proj, \
             tc.tile_pool(name="ps_score", bufs=2, space="PSUM") as ps_score, \
             tc.tile_pool(name="ps_av", bufs=2, space="PSUM") as ps_av:

            # ---- persistent SBUF state ----
            bq_sb = consts.tile([P, OC], f32)
            bk_sb = consts.tile([P, OC], f32)
            bv_sb = consts.tile([P, OC], f32)
            qT_sb = consts.tile([P, OC, T], bf16)    # [o, t] head-dim major
            kT_sb = consts.tile([P, OC, T], bf16)
            # v interleaved with ones: even heads [64 v | 64 ones],
            # odd heads [64 ones | 64 v]  (so denominators of a pair land on
            # opposite row halves of their AV psum tiles)
            v1_sb = consts.tile([P, HPG, KC, 128], bf16)
            outT_sb = consts.tile([P, OC, T], bf16)  # attn output^T [c, t]

            wq_sb = wpool.tile([P, DC, GD], bf16)
            wk_sb = wpool.tile([P, DC, GD], bf16)
            wv_sb = wpool.tile([P, DC, GD], bf16)
            wo_sb = wpool.tile([P, OC, D], bf16)

            # ---- ramp: one hwdge queue (DMA bandwidth is shared, so a
            # single need-ordered stream beats parallel queues), dc-pair
            # pieces interleaved so each projection's matmuls chase the
            # stream with ~0.5MB granularity ----
            xq0 = stage.tile([P, DC, 512], bf16, tag="xstage", name="xq0")
            xk0 = stage.tile([P, DC, 512], bf16, tag="xstage", name="xk0")
            xv0 = stage.tile([P, DC, 512], bf16, tag="xstage", name="xv0")
            for c in range(0, DC, 2):
                nc.sync.dma_start(wq_sb[:, c:c + 2, :], wq4[:, c:c + 2, :])
                nc.sync.dma_start(xq0[:, c:c + 2, :],
                                  xqT3[:, c:c + 2, 0:512])
            nc.sync.dma_start(bq_sb, bq2[:, :])
            for c in range(0, DC, 4):
                nc.sync.dma_start(wk_sb[:, c:c + 4, :], wk4[:, c:c + 4, :])
                nc.sync.dma_start(xk0[:, c:c + 4, :],
                                  xkT3[:, c:c + 4, 0:512])
            nc.sync.dma_start(bk_sb, bk2[:, :])
            for c in range(0, DC, 4):
                nc.sync.dma_start(wv_sb[:, c:c + 4, :], wv4[:, c:c + 4, :])
                nc.sync.dma_start(xv0[:, c:c + 4, :],
                                  xvT3[:, c:c + 4, 0:512])
            nc.sync.dma_start(bv_sb, bv2[:, :])
            nc.sync.dma_start(wo_sb, wo4[:, :, :])
            # ones blocks for the AV denominator trick (Pool, overlaps ramp)
            nc.gpsimd.memset(v1_sb[:, 0:HPG:2, :, 64:128], 1.0)
            nc.gpsimd.memset(v1_sb[:, 1:HPG:2, :, 0:64], 1.0)

            # ---- shared helpers ----
            def qk_proj_tile(w_sb, x_t, b_sb, dst, tt, oc):
                ps = ps_proj.tile([P, 512], f32, tag="proj", name="ps")
                for dc in range(DC):
                    nc.tensor.matmul(
                        ps, w_sb[:, dc, oc * P:(oc + 1) * P],
                        x_t[:, dc, :],
                        start=(dc == 0), stop=(dc == DC - 1))
                nc.vector.tensor_scalar_add(
                    dst[:, oc, tt * 512:(tt + 1) * 512], ps,
                    b_sb[:, oc:oc + 1])

            def v_proj_tile(x_t, tt, j):
                tch = tt * 4 + j
                ps = ps_proj.tile([P, GD], f32, tag="proj", name="ps")
                for dc in range(DC):
                    nc.tensor.matmul(
                        ps, x_t[:, dc, j * P:(j + 1) * P],
                        wv_sb[:, dc, :],
                        start=(dc == 0), stop=(dc == DC - 1))
                ps8 = ps.rearrange("p (h c) -> p h c", h=HPG)
                # even heads: v into cols 0:64; odd heads: cols 64:128
                nc.vector.tensor_copy(v1_sb[:, 0:HPG:2, tch, 0:64],
                                      ps8[:, 0:HPG:2, :])
                nc.vector.tensor_copy(v1_sb[:, 1:HPG:2, tch, 64:128],
                                      ps8[:, 1:HPG:2, :])

            def out_proj_half(tch, nh):
                # psum[t, o] = sum_cc outT[cc, t].T @ woT[cc, o]
                ps = ps_proj.tile([P, 512], f32, tag="proj", name="ps")
                for cc in range(OC):
                    nc.tensor.matmul(
                        ps, outT_sb[:, cc, tch * P:(tch + 1) * P],
                        wo_sb[:, cc, nh * 512:(nh + 1) * 512],
                        start=(cc == 0), stop=(cc == OC - 1))
                o_sb = outst.tile([P, 512], f32, tag="ost", name="o_sb")
                nc.vector.tensor_copy(o_sb, ps)
                nc.sync.dma_start(
                    out_p[tch * P:(tch + 1) * P, nh * 512:(nh + 1) * 512],
                    o_sb)

            # ---- attention: software-pipelined score->exp->mask->AV with a
            # 3-group lag; `pend` holds groups whose AV is not yet emitted.
            AV_LAG = 3
            pend = deque()
            draining = [False]

            epi_done = [0] * NQT   # epilogues emitted per qt

            def emit_av(e):
                blk = e["blk"]
                if blk["avA"] is None:
                    blk["avA"] = ps_av.tile([P, 512], f32, tag="av",
                                            name="avA")
                    blk["avB"] = ps_av.tile([P, 512], f32, tag="av",
                                            name="avB")
                n_k = blk["n_k"]
                # Head B first: B's deps (exB, and B's masks) have the latest
                # semaphore values, so the first-emitted matmul's wait covers
                # most of the rest (sems are monotonic) and they issue
                # wait-free. Within a psum bank, a start=True matmul must
                # execute first and the stop one last; plain accumulating
                # adds commute, so j-order only matters for start groups.
                jorder = [1, 0] if (len(e["kcs"]) == 2
                                    and e["kcs"][0] != 0
                                    and e["kcs"][-1] != n_k - 1) else \
                    list(range(len(e["kcs"])))
                for head in ("B", "A"):
                    av = blk["avB"] if head == "B" else blk["avA"]
                    h = blk["hB"] if head == "B" else blk["hA"]
                    ex = e["exB"] if head == "B" else e["exA"]
                    for j in jorder:
                        kc = e["kcs"][j]
                        qs = e["qss"][j]
                        nc.tensor.matmul(
                            av[:, qs:512], v1_sb[:, h, kc, :],
                            ex[:, j, qs:512],
                            start=(kc == 0), stop=(kc == n_k - 1))
                if e["last"]:
                    epilogue(blk, use_act=draining[0])

            def epilogue(blk, use_act=False):
                avA, avB = blk["avA"], blk["avB"]
                hoc, q0 = blk["hoc"], blk["q0"]
                # gather denominators: avA rows 64:128 -> den[0:64],
                #                      avB rows 0:64  -> den[64:128]
                den = small.tile([P, 512], f32, tag="den", name="den")
                nc.vector.tensor_copy(den[0:64, :], avA[64:128, :])
                nc.vector.tensor_copy(den[64:128, :], avB[0:64, :])
                rec = small.tile([P, 512], f32, tag="rec", name="rec")
                if use_act:
                    # drain-time path: ACT is idle then, and its Ln+Exp chain
                    # is shorter than the DVE Newton one (faster tail)
                    lnt = small.tile([P, 512], f32, tag="tnr", name="lnt")
                    nc.scalar.activation(lnt, den, Ln)
                    nc.scalar.activation(rec, lnt, Exp, scale=-1.0)
                else:
                    # reciprocal entirely on DVE (keeps ACT free for the exp
                    # stream): fast-inverse magic seed + one Newton step;
                    # seed err ~5% -> ~0.3% max after the step, ~0.1% rms —
                    # negligible vs the bf16 error floor here
                    u32 = mybir.dt.uint32
                    Alu = mybir.AluOpType
                    nc.vector.tensor_scalar(rec.bitcast(u32),
                                            den.bitcast(u32),
                                            0, None, Alu.bitwise_not)
                    nc.vector.tensor_scalar(rec.bitcast(u32),
                                            rec.bitcast(u32),
                                            0xFFFFFFFF - 0x7EF127EA, None,
                                            Alu.subtract)
                    tnr = small.tile([P, 512], f32, tag="tnr", name="tnr")
                    nc.vector.tensor_tensor(tnr, den, rec, Alu.mult)
                    nc.vector.tensor_scalar(tnr, tnr, -1.0, 2.0,
                                            Alu.mult, Alu.add)
                    nc.vector.tensor_tensor(rec, rec, tnr, Alu.mult)
                prod = small.tile([P, 512], f32, tag="prod", name="prod")
                nc.vector.tensor_tensor(prod[0:64, :], avA[0:64, :],
                                        rec[0:64, :], mybir.AluOpType.mult)
                nc.vector.tensor_tensor(prod[64:128, :], avB[64:128, :],
                                        rec[64:128, :], mybir.AluOpType.mult)
                nc.vector.tensor_scalar_add(
                    outT_sb[:, hoc, q0:q0 + 512], prod,
                    bv_sb[:, hoc:hoc + 1])
                epi_done[blk["qt"]] += 1

            def sc_group(blk, kcs, fill):
                qt, q0 = blk["qt"], blk["q0"]
                qss = [max(0, kc * P - q0) for kc in kcs]
                qsu = qss[0]
                hocA = blk["hoc"]
                kT_A = kT_sb[0:64, hocA, :]
                qT_A = qT_sb[0:64, hocA, :]
                kT_B = kT_sb[64:128, hocA, :]
                qT_B = qT_sb[64:128, hocA, :]
                scA = ps_score.tile([P, 2, 512], f32, tag="score", name="scA")
                scB = ps_score.tile([P, 2, 512], f32, tag="score", name="scB")
                # compute from the union window qsu for both chunks so the
                # exp below never reads unwritten psum (the sub-diagonal
                # [qsu, qs_j) cols are bounded garbage nothing reads back).
                # Both scA matmuls first: exp_A then starts while PE streams
                # scB, so the next group's scA WAR-wait is already satisfied
                # by the time PE reaches it (chain latency hidden).
                for j, kc in enumerate(kcs):
                    nc.tensor.matmul(
                        scA[:, j, qsu:512], kT_A[:, kc * P:(kc + 1) * P],
                        qT_A[:, q0 + qsu:q0 + 512], start=True, stop=True)
                for j, kc in enumerate(kcs):
                    nc.tensor.matmul(
                        scB[:, j, qsu:512], kT_B[:, kc * P:(kc + 1) * P],
                        qT_B[:, q0 + qsu:q0 + 512], start=True, stop=True)
                # drain one lagged AV group, then PE filler (proj tiles)
                while len(pend) >= AV_LAG + 1:
                    emit_av(pend.popleft())
                fill()
                exA = expp.tile([P, 2, 512], bf16, tag="exp", name="exA")
                nc.scalar.activation(exA[:, :, qsu:512], scA[:, :, qsu:512],
                                     Exp, scale=0.125)
                exB = expp.tile([P, 2, 512], bf16, tag="exp", name="exB")
                nc.scalar.activation(exB[:, :, qsu:512], scB[:, :, qsu:512],
                                     Exp, scale=0.125)
                # masks: all of A's then all of B's, so the B-first AV
                # emission's first wait covers A's mask sem values
                for ex in (exA, exB):
                    for j, kc in enumerate(kcs):
                        if kc >= 4 * qt:   # diagonal chunk -> mask
                            qs = qss[j]
                            nc.gpsimd.affine_select(
                                out=ex[:, j, qs:512],
                                in_=ex[:, j, qs:512],
                                compare_op=mybir.AluOpType.is_ge,
                                fill=0.0, base=q0 + qs - kc * P,
                                channel_multiplier=-1,
                                pattern=[[1, 512 - qs]])
                return {"kcs": kcs, "qss": qss, "exA": exA, "exB": exB,
                        "blk": blk, "last": False}

            # ---- phase plan ----
            # tt0 projections solid (DMA-gated ramp), then per qt phase:
            # attention over 4 pairs with proj(tt+1) + out_proj(qt-1)
            # matmuls as filler.
            x_tiles = {0: {"q": xq0}}
            deferred = []   # gated fillers that could not run in their phase
            reserve = []    # fillers held back to cover the drain bubble

            def stage_x(tt, key, src3):
                t_ = stage.tile([P, DC, 512], bf16, tag="xstage",
                                name=f"x{key}{tt}")
                nc.gpsimd.dma_start(t_, src3[:, :, tt * 512:(tt + 1) * 512])
                x_tiles.setdefault(tt, {})[key] = t_
                return t_

            # tt0 projections: emit only what attention(qt0, pair0) needs
            # solid (Q tiles chase their DMA pieces anyway; K-oc0, V j0/j1);
            # the rest become early fillers inside the qt0 phase so PE/ACT
            # start attention while the K/V ramp DMAs are still landing
            for oc in range(OC):
                qk_proj_tile(wq_sb, xq0, bq_sb, qT_sb, 0, oc)
            qk_proj_tile(wk_sb, xk0, bk_sb, kT_sb, 0, 0)
            v_proj_tile(xv0, 0, 0)
            v_proj_tile(xv0, 0, 1)
            ramp_left = [
                lambda: qk_proj_tile(wk_sb, xk0, bk_sb, kT_sb, 0, 1),
                lambda: v_proj_tile(xv0, 0, 2),
                lambda: v_proj_tile(xv0, 0, 3),
                lambda: qk_proj_tile(wk_sb, xk0, bk_sb, kT_sb, 0, 2),
                lambda: qk_proj_tile(wk_sb, xk0, bk_sb, kT_sb, 0, 3),
            ]

            for qt in range(NQT):
                # stage next tile's x + build filler queue
                fillers = []
                if qt == 0:
                    fillers.extend(ramp_left)
                if qt + 1 < NQT:
                    tt = qt + 1
                    xq_t = stage_x(tt, "q", xqT3)
                    xk_t = stage_x(tt, "k", xkT3)
                    xv_t = stage_x(tt, "v", xvT3)
                    for oc in range(OC):
                        fillers.append(
                            lambda oc=oc, tt=tt, x=xq_t: qk_proj_tile(
                                wq_sb, x, bq_sb, qT_sb, tt, oc))
                    for oc in range(OC):
                        fillers.append(
                            lambda oc=oc, tt=tt, x=xk_t: qk_proj_tile(
                                wk_sb, x, bk_sb, kT_sb, tt, oc))
                    for j in range(4):
                        fillers.append(
                            lambda j=j, tt=tt, x=xv_t: v_proj_tile(x, tt, j))
                # out_proj fillers: qt1 takes q0's, qt3 takes q1's AND q2's
                # (qt3 has 32 groups but no projection fillers left, while
                # qt2 still carries proj(tt3)); 4 halves held in reserve for
                # the drain bubble
                op_qts = {1: [0], 3: [1, 2]}.get(qt, [])
                for oqt in op_qts:
                    for tch in range(4 * oqt, 4 * oqt + 4):
                        for nh in range(2):
                            # gated: outT cols for oqt must be complete
                            f = (oqt, lambda tch=tch, nh=nh:
                                 out_proj_half(tch, nh))
                            if qt == NQT - 1 and oqt == 2 and tch >= 9:
                                reserve.append(f)
                            else:
                                fillers.append(f)

                n_groups = 4 * 2 * (qt + 1)   # pairs * groups-per-pair
                gctr = [0]
                popped = [0]

                def run_filler(f):
                    if isinstance(f, tuple):
                        gate_qt, fn = f
                        if epi_done[gate_qt] < 4:
                            return False
                        fn()
                    else:
                        f()
                    return True

                def fill():
                    gctr[0] += 1
                    want = len(fillers) * gctr[0] // n_groups
                    while popped[0] < min(want, len(fillers)):
                        if not run_filler(fillers[popped[0]]):
                            break
                        popped[0] += 1

                n_k = 4 * (qt + 1)
                for pair in range(HPG // 2):
                    blk = {"qt": qt, "q0": qt * 512, "n_k": n_k,
                           "hA": 2 * pair, "hB": 2 * pair + 1, "hoc": pair,
                           "avA": None, "avB": None}
                    entries = []
                    for g in range(n_k // 2):
                        e = sc_group(blk, (2 * g, 2 * g + 1), fill)
                        entries.append(e)
                        pend.append(e)
                    entries[-1]["last"] = True
                # leftover fillers: emit any whose gate is satisfied now;
                # truly-blocked ones (last qt's out_proj) defer to the drain
                leftovers = []
                while popped[0] < len(fillers):
                    f = fillers[popped[0]]
                    if not run_filler(f):
                        leftovers.append(f)
                    popped[0] += 1
                deferred.extend(leftovers)

            # drain the pipeline; the reserved PE work goes AFTER the last
            # epilogue so PE streams out-proj tiles while the final DVE
            # divide chain completes (the last out_proj batch depends on it)
            draining[0] = True
            while pend:
                emit_av(pend.popleft())
                if len(pend) == 2 and reserve:
                    assert run_filler(reserve.pop())
            for f in reserve:
                assert run_filler(f)
            for f in deferred:
                assert run_filler(f)
            for tch in range(12, KC):
                for nh in range(2):
                    out_proj_half(tch, nh)

    _nc_cache = nc
    return nc


# ---------------------------------------------------------------------------
# host wrapper
# ---------------------------------------------------------------------------
def _shard_inputs(inputs):
    q, k, v = inputs["query"], inputs["key"], inputs["value"]
    in_maps = []
    for core in range(N_CORES):
        b, g = core // G, core % G
        gs, ge = g * GD, (g + 1) * GD
        m = {
            "xqT": np.ascontiguousarray(q[b].T).astype(BF16),
            "xkT": np.ascontiguousarray(k[b].T).astype(BF16),
            "xvT": np.ascontiguousarray(v[b].T).astype(BF16),
            "wqT": np.ascontiguousarray(inputs["Wq"][gs:ge, :].T).astype(BF16),
            "wkT": np.ascontiguousarray(inputs["Wk"][gs:ge, :].T).astype(BF16),
            "wvT": np.ascontiguousarray(inputs["Wv"][gs:ge, :].T).astype(BF16),
            "woT": np.ascontiguousarray(inputs["Wo"][:, gs:ge].T).astype(BF16),
            "bq2": np.ascontiguousarray(
                inputs["bq"][gs:ge].reshape(OC, P).T).astype(np.float32),
            "bk2": np.ascontiguousarray(
                inputs["bk"][gs:ge].reshape(OC, P).T).astype(np.float32),
            "bv2": np.ascontiguousarray(
                inputs["bv"][gs:ge].reshape(OC, P).T).astype(np.float32),
        }
        in_maps.append(m)
    return in_maps


def run_spmd(inputs, trace=False, **kw):
    """Returns (BassKernelResults, combined_output)."""
    _patch_compile()
    from concourse.bass_utils import run_bass_kernel_spmd
    nc = build_nc()
    in_maps = _shard_inputs(inputs)
    res = run_bass_kernel_spmd(nc, in_maps, core_ids=list(range(N_CORES)),
                               trace=trace, **kw)
    bo = inputs["bo"].astype(np.float32)
    out = np.empty((B, T, D), dtype=np.float32)
    for b in range(B):
        out[b] = res.results[2 * b]["out_p"] + res.results[2 * b + 1]["out_p"] + bo
    return res, out


def kernel(**inputs) -> np.ndarray:
    _, out = run_spmd(inputs, trace=False)
    return out


# revision 44
# speedup vs baseline: 1.2432x; 1.2432x over previous
"""Causal multi-head attention layer on 8 trn2 NeuronCores.

Sharding: 8 shards = 4 batches x 2 head-groups (8 heads each).
Each core computes, for its (batch b, head-group g):
  - Q/K projections transposed: qT/kT [512, 2048] (head-dim on partitions)
  - V projection in natural layout [2048, 512], stored interleaved with an
    all-ones block (even heads [v|ones], odd heads [ones|v]) so the AV
    matmul yields both sum(p*v) and sum(p) at once, with the two heads of a
    pair landing their denominators on opposite row halves
  - causal attention per head with scores computed transposed (S^T = K Q^T),
    exp on ScalarE (scale=1/8 folded in), no max subtraction (scores bounded
    for this input distribution)
  - softmax divide: both heads' denominators gathered into one [128,512]
    tile (2 DVE copies), one Ln+Exp pair on ScalarE, products on DVE
  - partial output projection (row-split): out_part = attn_g @ Wo[:, g].T
Host gathers: out[b] = out_part[b,g=0] + out_part[b,g=1] + bo.

Schedule: the attention score->exp->mask->AV chain is software-pipelined
with a 2-group lag (scores run ahead; AV matmuls trail by two groups), and
projection / output-projection tiles are emitted as PE filler inside the
ACT-paced attention stream so the tensor engine never idles.

All matmuls in bf16 (fp32 accumulation in PSUM).
"""
import os
from collections import deque
import numpy as np
import ml_dtypes

B, T, D = 4, 2048, 1024
NH, HD = 16, 64
G = 2                 # head groups
HPG = NH // G         # heads per core = 8
GD = D // G           # group dim = 512
P = 128
DC = D // P           # 8  d-chunks
OC = GD // P          # 4  output chunks per group
KC = T // P           # 16 k chunks
NQT = T // 512        # 4  q tiles
N_CORES = 8

BF16 = ml_dtypes.bfloat16
E4M3 = ml_dtypes.float8_e4m3fn

# ---------------------------------------------------------------------------
# BIR compatibility patch: the bundled walrus rejects two bass encodings.
# 1) EVENT_SEMAPHORE_RANGE_CLEAR raw-ISA -> rewrite to per-sem EventSemaphore
#    writes (sem-wr-imm 0, the legacy reset mechanism).
# 2) >1 sync-wait per instruction -> hoist extras onto preceding NoOps.
# ---------------------------------------------------------------------------
_patched = False


def _fix_bir(bir_json: bytes) -> bytes:
    import orjson
    bir = orjson.loads(bir_json)
    changed = False
    for fn in bir.get("functions", []):
        for blk in fn.get("blocks", []):
            new_insts = []
            for inst in blk.get("instructions", []):
                si = inst.get("sync_info") or {}
                waits = si.get("on_wait") or []
                if len(waits) > 1 and inst.get("opcode") != "ISA":
                    changed = True
                    for i, w in enumerate(waits[1:]):
                        new_insts.append({
                            "opcode": "NoOp",
                            "name": f"{inst['name']}_wait{i}",
                            "engine": inst.get("engine", "Pool"),
                            "debug": inst.get("debug"),
                            "ins": [], "outs": [],
                            "sync_info": {"on_update": [], "on_wait": [w]},
                        })
                    si = dict(si)
                    si["on_wait"] = waits[:1]
                    inst = dict(inst)
                    inst["sync_info"] = si
                if (inst.get("opcode") == "ISA"
                        and inst.get("op_name") == "EVENT_SEMAPHORE_RANGE_CLEAR"):
                    ad = inst.get("ant_dict", {})
                    changed = True
                    # round-robin the per-sem clears across all engines: they
                    # run before everything, so spreading keeps each engine's
                    # share ~3us instead of serializing on one queue
                    engines = ["Pool", "Activation", "DVE", "PE", "SP"]
                    orig_wait = (inst.get("sync_info") or {}).get("on_wait", [])
                    seen_eng = set()
                    for i, sem in enumerate(range(ad["range_first"],
                                                  ad["range_last"] + 1)):
                        eng = engines[i % len(engines)]
                        first_on_eng = eng not in seen_eng
                        seen_eng.add(eng)
                        new_insts.append({
                            "opcode": "EventSemaphore",
                            "name": f"{inst['name']}_wr{i}",
                            "engine": eng,
                            "debug": inst.get("debug"),
                            "ins": [], "outs": [],
                            "sync_info": {
                                "on_update": [{
                                    "sync_type": "semaphore", "id": sem,
                                    "update_mode": "sem-wr-imm",
                                    "update_value": 0,
                                    "ant_name": f"semclear_{sem}",
                                }],
                                "on_wait": orig_wait if first_on_eng else [],
                            },
                        })
                else:
                    new_insts.append(inst)
            blk["instructions"] = new_insts
    return orjson.dumps(bir) if changed else bir_json


def _patch_compile():
    global _patched
    if _patched:
        return
    _patched = True
    import concourse.bass_utils as bu
    import concourse.bass2jax as b2j
    orig = bu.compile_bir_kernel

    def wrapped(bir_json, tmpdir, neff_name="file.neff"):
        return orig(_fix_bir(bir_json), tmpdir, neff_name)

    bu.compile_bir_kernel = wrapped
    if hasattr(b2j, "compile_bir_kernel"):
        b2j.compile_bir_kernel = wrapped

    # no-egress sandbox: skip artifact upload in the trace path
    bu.upload_artifacts = lambda tmpdir: f"local:{tmpdir}"

    # provide antenv.axon_hooks (missing in this image) so trace=True can
    # reach the NTFF profiling hook in libaxon_pjrt.so
    import sys as _sys
    import types as _types
    if "antenv.axon_hooks" not in _sys.modules:
        try:
            import antenv
            mod = _types.ModuleType("antenv.axon_hooks")
            mod._hook = None
            mod.set_axon_ntff_profile_hook = lambda h: setattr(mod, "_hook", h)
            mod.get_axon_ntff_profile_hook = lambda: mod._hook
            _sys.modules["antenv.axon_hooks"] = mod
            antenv.axon_hooks = mod
            so_path = "/opt/axon/libaxon_pjrt.so"
            if os.path.exists(so_path):
                from trn_agent_boot.trn_boot import _ntff_profile_via_ctypes
                mod._hook = _ntff_profile_via_ctypes(so_path)
        except Exception:
            pass


# ---------------------------------------------------------------------------
# Bass program (identical on all 8 cores; data differs per core)
# ---------------------------------------------------------------------------
_nc_cache = None


def build_nc():
    global _nc_cache
    if _nc_cache is not None:
        return _nc_cache
    import concourse.bass as bass
    import concourse.mybir as mybir
    from concourse.tile import TileContext

    f32 = mybir.dt.float32
    bf16 = mybir.dt.bfloat16
    Exp = mybir.ActivationFunctionType.Exp
    Ln = mybir.ActivationFunctionType.Ln

    fp8 = mybir.dt.float8e4
    DR = mybir.MatmulPerfMode.DoubleRow

    nc = bass.Bass()
    # Q/K paths run in fp8e4 (x direct, W pre-scaled by 64 on the host so
    # the weights sit in e4m3's normal range; 1/64 folded into the bias
    # add). The fp8 DoubleRow matmul contracts two 128-k-tiles per
    # instruction, halving the Q/K projection instruction count.
    xqT = nc.dram_tensor("xqT", [D, T], fp8, kind="ExternalInput")
    xkT = nc.dram_tensor("xkT", [D, T], fp8, kind="ExternalInput")
    xvT = nc.dram_tensor("xvT", [D, T], bf16, kind="ExternalInput")
    wqT = nc.dram_tensor("wqT", [D, GD], fp8, kind="ExternalInput")
    wkT = nc.dram_tensor("wkT", [D, GD], fp8, kind="ExternalInput")
    wvT = nc.dram_tensor("wvT", [D, GD], bf16, kind="ExternalInput")
    woT = nc.dram_tensor("woT", [GD, D], bf16, kind="ExternalInput")
    bq2 = nc.dram_tensor("bq2", [P, OC], f32, kind="ExternalInput")
    bk2 = nc.dram_tensor("bk2", [P, OC], f32, kind="ExternalInput")
    bv2 = nc.dram_tensor("bv2", [P, OC], f32, kind="ExternalInput")
    out_p = nc.dram_tensor("out_p", [T, D], f32, kind="ExternalOutput")

    xqT3 = xqT.rearrange("(dc p) t -> p dc t", p=P)
    xkT3 = xkT.rearrange("(dc p) t -> p dc t", p=P)
    xvT3 = xvT.rearrange("(dc p) t -> p dc t", p=P)
    wq4 = wqT.rearrange("(dc p) o -> p dc o", p=P)
    wk4 = wkT.rearrange("(dc p) o -> p dc o", p=P)
    wv4 = wvT.rearrange("(dc p) o -> p dc o", p=P)
    wo4 = woT.rearrange("(cc p) o -> p cc o", p=P)

    with TileContext(nc) as tc:
        with tc.tile_pool(name="consts", bufs=1) as consts, \
             tc.tile_pool(name="wpool", bufs=1) as wpool, \
             tc.tile_pool(name="stage", bufs=4) as stage, \
             tc.tile_pool(name="expp", bufs=8) as expp, \
             tc.tile_pool(name="small", bufs=2) as small, \
             tc.tile_pool(name="outst", bufs=4) as outst, \
             tc.tile_pool(name="ps_proj", bufs=2, space="PSUM") as ps_proj, \
             tc.tile_pool(name="ps_score", bufs=2, space="PSUM") as ps_score, \
             tc.tile_pool(name="ps_av", bufs=2, space="PSUM") as ps_av:

            # ---- persistent SBUF state ----
            bq_sb = consts.tile([P, OC], f32)
            bk_sb = consts.tile([P, OC], f32)
            bv_sb = consts.tile([P, OC], f32)
            qT_sb = consts.tile([P, OC, T], bf16)    # [o, t] head-dim major
            kT_sb = consts.tile([P, OC, T], bf16)
            # v interleaved with ones: even heads [64 v | 64 ones],
            # odd heads [64 ones | 64 v]  (so denominators of a pair land on
            # opposite row halves of their AV psum tiles)
            v1_sb = consts.tile([P, HPG, KC, 128], bf16)
            outT_sb = consts.tile([P, OC, T], bf16)  # attn output^T [c, t]

            wq_sb = wpool.tile([P, DC, GD], fp8)
            wk_sb = wpool.tile([P, DC, GD], fp8)
            wv_sb = wpool.tile([P, DC, GD], bf16)
            wo_sb = wpool.tile([P, OC, D], bf16)

            # ---- ramp: one hwdge queue (DMA bandwidth is shared, so a
            # single need-ordered stream beats parallel queues), dc-pair
            # pieces interleaved so each projection's matmuls chase the
            # stream with ~0.5MB granularity ----
            xq0 = stage.tile([P, DC, 512], fp8, tag="x8stage", name="xq0")
            xk0 = stage.tile([P, DC, 512], fp8, tag="x8stage", name="xk0")
            xv0 = stage.tile([P, DC, 512], bf16, tag="xstage", name="xv0")
            for c in range(0, DC, 2):
                nc.sync.dma_start(wq_sb[:, c:c + 2, :], wq4[:, c:c + 2, :])
                nc.sync.dma_start(xq0[:, c:c + 2, :],
                                  xqT3[:, c:c + 2, 0:512])
            nc.sync.dma_start(bq_sb, bq2[:, :])
            for c in range(0, DC, 4):
                nc.sync.dma_start(wk_sb[:, c:c + 4, :], wk4[:, c:c + 4, :])
                nc.sync.dma_start(xk0[:, c:c + 4, :],
                                  xkT3[:, c:c + 4, 0:512])
            nc.sync.dma_start(bk_sb, bk2[:, :])
            for c in range(0, DC, 4):
                nc.sync.dma_start(wv_sb[:, c:c + 4, :], wv4[:, c:c + 4, :])
                nc.sync.dma_start(xv0[:, c:c + 4, :],
                                  xvT3[:, c:c + 4, 0:512])
            nc.sync.dma_start(bv_sb, bv2[:, :])
            nc.sync.dma_start(wo_sb, wo4[:, :, :])
            # ones blocks for the AV denominator trick (Pool, overlaps ramp)
            nc.gpsimd.memset(v1_sb[:, 0:HPG:2, :, 64:128], 1.0)
            nc.gpsimd.memset(v1_sb[:, 1:HPG:2, :, 0:64], 1.0)

            # ---- shared helpers ----
            def qk_proj_tile(w_sb, x_t, b_sb, dst, tt, oc):
                # fp8 DoubleRow: 4 dc-pair matmuls, 64x weight scale undone
                # in the bias-add evacuation
                ps = ps_proj.tile([P, 512], f32, tag="proj", name="ps")
                for dp in range(DC // 2):
                    nc.tensor.matmul(
                        ps, w_sb[:, 2 * dp:2 * dp + 2,
                                 oc * P:(oc + 1) * P],
                        x_t[:, 2 * dp:2 * dp + 2, :],
                        start=(dp == 0), stop=(dp == DC // 2 - 1),
                        perf_mode=DR)
                nc.vector.tensor_scalar(
                    dst[:, oc, tt * 512:(tt + 1) * 512], ps,
                    1.0 / 64.0, b_sb[:, oc:oc + 1],
                    mybir.AluOpType.mult, mybir.AluOpType.add)

            def v_proj_tile(x_t, tt, j):
                tch = tt * 4 + j
                ps = ps_proj.tile([P, GD], f32, tag="proj", name="ps")
                for dc in range(DC):
                    nc.tensor.matmul(
                        ps, x_t[:, dc, j * P:(j + 1) * P],
                        wv_sb[:, dc, :],
                        start=(dc == 0), stop=(dc == DC - 1))
                ps8 = ps.rearrange("p (h c) -> p h c", h=HPG)
                # even heads: v into cols 0:64; odd heads: cols 64:128
                nc.vector.tensor_copy(v1_sb[:, 0:HPG:2, tch, 0:64],
                                      ps8[:, 0:HPG:2, :])
                nc.vector.tensor_copy(v1_sb[:, 1:HPG:2, tch, 64:128],
                                      ps8[:, 1:HPG:2, :])

            def out_proj_half(tch, nh):
                # psum[t, o] = sum_cc outT[cc, t].T @ woT[cc, o]
                ps = ps_proj.tile([P, 512], f32, tag="proj", name="ps")
                for cc in range(OC):
                    nc.tensor.matmul(
                        ps, outT_sb[:, cc, tch * P:(tch + 1) * P],
                        wo_sb[:, cc, nh * 512:(nh + 1) * 512],
                        start=(cc == 0), stop=(cc == OC - 1))
                o_sb = outst.tile([P, 512], f32, tag="ost", name="o_sb")
                nc.vector.tensor_copy(o_sb, ps)
                nc.sync.dma_start(
                    out_p[tch * P:(tch + 1) * P, nh * 512:(nh + 1) * 512],
                    o_sb)

            # ---- attention: software-pipelined score->exp->mask->AV with a
            # 3-group lag; `pend` holds groups whose AV is not yet emitted.
            AV_LAG = 3
            pend = deque()
            draining = [False]

            epi_done = [0] * NQT   # epilogues emitted per qt

            def emit_av(e):
                blk = e["blk"]
                if blk["avA"] is None:
                    blk["avA"] = ps_av.tile([P, 512], f32, tag="av",
                                            name="avA")
                    blk["avB"] = ps_av.tile([P, 512], f32, tag="av",
                                            name="avB")
                n_k = blk["n_k"]
                # Head B first: B's deps (exB, and B's masks) have the latest
                # semaphore values, so the first-emitted matmul's wait covers
                # most of the rest (sems are monotonic) and they issue
                # wait-free. Within a psum bank, a start=True matmul must
                # execute first and the stop one last; plain accumulating
                # adds commute, so j-order only matters for start groups.
                jorder = [1, 0] if (len(e["kcs"]) == 2
                                    and e["kcs"][0] != 0
                                    and e["kcs"][-1] != n_k - 1) else \
                    list(range(len(e["kcs"])))
                for head in ("B", "A"):
                    av = blk["avB"] if head == "B" else blk["avA"]
                    h = blk["hB"] if head == "B" else blk["hA"]
                    ex = e["exB"] if head == "B" else e["exA"]
                    for j in jorder:
                        kc = e["kcs"][j]
                        qs = e["qss"][j]
                        nc.tensor.matmul(
                            av[:, qs:512], v1_sb[:, h, kc, :],
                            ex[:, j, qs:512],
                            start=(kc == 0), stop=(kc == n_k - 1))
                if e["last"]:
                    epilogue(blk, use_act=draining[0])

            def epilogue(blk, use_act=False):
                avA, avB = blk["avA"], blk["avB"]
                hoc, q0 = blk["hoc"], blk["q0"]
                # gather denominators: avA rows 64:128 -> den[0:64],
                #                      avB rows 0:64  -> den[64:128]
                den = small.tile([P, 512], f32, tag="den", name="den")
                nc.vector.tensor_copy(den[0:64, :], avA[64:128, :])
                nc.vector.tensor_copy(den[64:128, :], avB[0:64, :])
                rec = small.tile([P, 512], f32, tag="rec", name="rec")
                if use_act:
                    # drain-time path: ACT is idle then, and its Ln+Exp chain
                    # is shorter than the DVE Newton one (faster tail)
                    lnt = small.tile([P, 512], f32, tag="tnr", name="lnt")
                    nc.scalar.activation(lnt, den, Ln)
                    nc.scalar.activation(rec, lnt, Exp, scale=-1.0)
                else:
                    # reciprocal entirely on DVE (keeps ACT free for the exp
                    # stream): fast-inverse magic seed + one Newton step;
                    # seed err ~5% -> ~0.3% max after the step, ~0.1% rms —
                    # negligible vs the bf16 error floor here
                    u32 = mybir.dt.uint32
                    Alu = mybir.AluOpType
                    nc.vector.tensor_scalar(rec.bitcast(u32),
                                            den.bitcast(u32),
                                            0, None, Alu.bitwise_not)
                    nc.vector.tensor_scalar(rec.bitcast(u32),
                                            rec.bitcast(u32),
                                            0xFFFFFFFF - 0x7EF127EA, None,
                                            Alu.subtract)
                    tnr = small.tile([P, 512], f32, tag="tnr", name="tnr")
                    nc.vector.tensor_tensor(tnr, den, rec, Alu.mult)
                    nc.vector.tensor_scalar(tnr, tnr, -1.0, 2.0,
                                            Alu.mult, Alu.add)
                    nc.vector.tensor_tensor(rec, rec, tnr, Alu.mult)
                prod = small.tile([P, 512], f32, tag="prod", name="prod")
                nc.vector.tensor_tensor(prod[0:64, :], avA[0:64, :],
                                        rec[0:64, :], mybir.AluOpType.mult)
                nc.vector.tensor_tensor(prod[64:128, :], avB[64:128, :],
                                        rec[64:128, :], mybir.AluOpType.mult)
                nc.vector.tensor_scalar_add(
                    outT_sb[:, hoc, q0:q0 + 512], prod,
                    bv_sb[:, hoc:hoc + 1])
                epi_done[blk["qt"]] += 1

            def sc_group(blk, kcs, fill):
                qt, q0 = blk["qt"], blk["q0"]
                qss = [max(0, kc * P - q0) for kc in kcs]
                qsu = qss[0]
                hocA = blk["hoc"]
                kT_A = kT_sb[0:64, hocA, :]
                qT_A = qT_sb[0:64, hocA, :]
                kT_B = kT_sb[64:128, hocA, :]
                qT_B = qT_sb[64:128, hocA, :]
                scA = ps_score.tile([P, 2, 512], f32, tag="score", name="scA")
                scB = ps_score.tile([P, 2, 512], f32, tag="score", name="scB")
                # compute from the union window qsu for both chunks so the
                # exp below never reads unwritten psum (the sub-diagonal
                # [qsu, qs_j) cols are bounded garbage nothing reads back).
                # Both scA matmuls first: exp_A then starts while PE streams
                # scB, so the next group's scA WAR-wait is already satisfied
                # by the time PE reaches it (chain latency hidden).
                for j, kc in enumerate(kcs):
                    nc.tensor.matmul(
                        scA[:, j, qsu:512], kT_A[:, kc * P:(kc + 1) * P],
                        qT_A[:, q0 + qsu:q0 + 512], start=True, stop=True)
                for j, kc in enumerate(kcs):
                    nc.tensor.matmul(
                        scB[:, j, qsu:512], kT_B[:, kc * P:(kc + 1) * P],
                        qT_B[:, q0 + qsu:q0 + 512], start=True, stop=True)
                # drain one lagged AV group, then PE filler (proj tiles)
                while len(pend) >= AV_LAG + 1:
                    emit_av(pend.popleft())
                fill()
                exA = expp.tile([P, 2, 512], bf16, tag="exp", name="exA")
                nc.scalar.activation(exA[:, :, qsu:512], scA[:, :, qsu:512],
                                     Exp, scale=0.125)
                exB = expp.tile([P, 2, 512], bf16, tag="exp", name="exB")
                nc.scalar.activation(exB[:, :, qsu:512], scB[:, :, qsu:512],
                                     Exp, scale=0.125)
                # masks: all of A's then all of B's, so the B-first AV
                # emission's first wait covers A's mask sem values
                for ex in (exA, exB):
                    for j, kc in enumerate(kcs):
                        if kc >= 4 * qt:   # diagonal chunk -> mask
                            qs = qss[j]
                            nc.gpsimd.affine_select(
                                out=ex[:, j, qs:512],
                                in_=ex[:, j, qs:512],
                                compare_op=mybir.AluOpType.is_ge,
                                fill=0.0, base=q0 + qs - kc * P,
                                channel_multiplier=-1,
                                pattern=[[1, 512 - qs]])
                return {"kcs": kcs, "qss": qss, "exA": exA, "exB": exB,
                        "blk": blk, "last": False}

            # ---- phase plan ----
            # tt0 projections solid (DMA-gated ramp), then per qt phase:
            # attention over 4 pairs with proj(tt+1) + out_proj(qt-1)
            # matmuls as filler.
            x_tiles = {0: {"q": xq0}}
            deferred = []   # gated fillers that could not run in their phase
            reserve = []    # fillers held back to cover the drain bubble

            def stage_x(tt, key, src3):
                dt_, tag = (fp8, "x8stage") if key in ("q", "k") \
                    else (bf16, "xstage")
                t_ = stage.tile([P, DC, 512], dt_, tag=tag,
                                name=f"x{key}{tt}")
                nc.gpsimd.dma_start(t_, src3[:, :, tt * 512:(tt + 1) * 512])
                x_tiles.setdefault(tt, {})[key] = t_
                return t_

            # tt0 projections: emit only what attention(qt0, pair0) needs
            # solid (Q tiles chase their DMA pieces anyway; K-oc0, V j0/j1);
            # the rest become early fillers inside the qt0 phase so PE/ACT
            # start attention while the K/V ramp DMAs are still landing
            for oc in range(OC):
                qk_proj_tile(wq_sb, xq0, bq_sb, qT_sb, 0, oc)
            qk_proj_tile(wk_sb, xk0, bk_sb, kT_sb, 0, 0)
            v_proj_tile(xv0, 0, 0)
            v_proj_tile(xv0, 0, 1)
            ramp_left = [
                lambda: qk_proj_tile(wk_sb, xk0, bk_sb, kT_sb, 0, 1),
                lambda: v_proj_tile(xv0, 0, 2),
                lambda: v_proj_tile(xv0, 0, 3),
                lambda: qk_proj_tile(wk_sb, xk0, bk_sb, kT_sb, 0, 2),
                lambda: qk_proj_tile(wk_sb, xk0, bk_sb, kT_sb, 0, 3),
            ]

            for qt in range(NQT):
                # stage next tile's x + build filler queue
                fillers = []
                if qt == 0:
                    fillers.extend(ramp_left)
                if qt + 1 < NQT:
                    tt = qt + 1
                    xq_t = stage_x(tt, "q", xqT3)
                    xk_t = stage_x(tt, "k", xkT3)
                    xv_t = stage_x(tt, "v", xvT3)
                    for oc in range(OC):
                        fillers.append(
                            lambda oc=oc, tt=tt, x=xq_t: qk_proj_tile(
                                wq_sb, x, bq_sb, qT_sb, tt, oc))
                    for oc in range(OC):
                        fillers.append(
                            lambda oc=oc, tt=tt, x=xk_t: qk_proj_tile(
                                wk_sb, x, bk_sb, kT_sb, tt, oc))
                    for j in range(4):
                        fillers.append(
                            lambda j=j, tt=tt, x=xv_t: v_proj_tile(x, tt, j))
                # out_proj fillers: qt1 takes q0's, qt3 takes q1's AND q2's
                # (qt3 has 32 groups but no projection fillers left, while
                # qt2 still carries proj(tt3)); 4 halves held in reserve for
                # the drain bubble
                op_qts = {1: [0], 3: [1, 2]}.get(qt, [])
                for oqt in op_qts:
                    for tch in range(4 * oqt, 4 * oqt + 4):
                        for nh in range(2):
                            # gated: outT cols for oqt must be complete
                            f = (oqt, lambda tch=tch, nh=nh:
                                 out_proj_half(tch, nh))
                            if qt == NQT - 1 and oqt == 2 and tch >= 9:
                                reserve.append(f)
                            else:
                                fillers.append(f)

                n_groups = 4 * 2 * (qt + 1)   # pairs * groups-per-pair
                gctr = [0]
                popped = [0]

                def run_filler(f):
                    if isinstance(f, tuple):
                        gate_qt, fn = f
                        if epi_done[gate_qt] < 4:
                            return False
                        fn()
                    else:
                        f()
                    return True

                def fill():
                    gctr[0] += 1
                    want = len(fillers) * gctr[0] // n_groups
                    while popped[0] < min(want, len(fillers)):
                        if not run_filler(fillers[popped[0]]):
                            break
                        popped[0] += 1

                n_k = 4 * (qt + 1)
                for pair in range(HPG // 2):
                    blk = {"qt": qt, "q0": qt * 512, "n_k": n_k,
                           "hA": 2 * pair, "hB": 2 * pair + 1, "hoc": pair,
                           "avA": None, "avB": None}
                    entries = []
                    for g in range(n_k // 2):
                        e = sc_group(blk, (2 * g, 2 * g + 1), fill)
                        entries.append(e)
                        pend.append(e)
                    entries[-1]["last"] = True
                # leftover fillers: emit any whose gate is satisfied now;
                # truly-blocked ones (last qt's out_proj) defer to the drain
                leftovers = []
                while popped[0] < len(fillers):
                    f = fillers[popped[0]]
                    if not run_filler(f):
                        leftovers.append(f)
                    popped[0] += 1
                deferred.extend(leftovers)

            # drain the pipeline; the reserved PE work goes AFTER the last
            # epilogue so PE streams out-proj tiles while the final DVE
            # divide chain completes (the last out_proj batch depends on it)
            draining[0] = True
            while pend:
                emit_av(pend.popleft())
                if len(pend) == 2 and reserve:
                    assert run_filler(reserve.pop())
            for f in reserve:
                assert run_filler(f)
            for f in deferred:
                assert run_filler(f)
            for tch in range(12, KC):
                for nh in range(2):
                    out_proj_half(tch, nh)

    _nc_cache = nc
    return nc


# ---------------------------------------------------------------------------
# host wrapper
# ---------------------------------------------------------------------------
def _shard_inputs(inputs):
    q, k, v = inputs["query"], inputs["key"], inputs["value"]
    in_maps = []
    for core in range(N_CORES):
        b, g = core // G, core % G
        gs, ge = g * GD, (g + 1) * GD
        m = {
            "xqT": np.ascontiguousarray(q[b].T).astype(E4M3),
            "xkT": np.ascontiguousarray(k[b].T).astype(E4M3),
            "xvT": np.ascontiguousarray(v[b].T).astype(BF16),
            "wqT": np.ascontiguousarray(
                inputs["Wq"][gs:ge, :].T * 64.0).astype(E4M3),
            "wkT": np.ascontiguousarray(
                inputs["Wk"][gs:ge, :].T * 64.0).astype(E4M3),
            "wvT": np.ascontiguousarray(inputs["Wv"][gs:ge, :].T).astype(BF16),
            "woT": np.ascontiguousarray(inputs["Wo"][:, gs:ge].T).astype(BF16),
            "bq2": np.ascontiguousarray(
                inputs["bq"][gs:ge].reshape(OC, P).T).astype(np.float32),
            "bk2": np.ascontiguousarray(
                inputs["bk"][gs:ge].reshape(OC, P).T).astype(np.float32),
            "bv2": np.ascontiguousarray(
                inputs["bv"][gs:ge].reshape(OC, P).T).astype(np.float32),
        }
        in_maps.append(m)
    return in_maps


def run_spmd(inputs, trace=False, **kw):
    """Returns (BassKernelResults, combined_output)."""
    _patch_compile()
    from concourse.bass_utils import run_bass_kernel_spmd
    nc = build_nc()
    in_maps = _shard_inputs(inputs)
    res = run_bass_kernel_spmd(nc, in_maps, core_ids=list(range(N_CORES)),
                               trace=trace, **kw)
    bo = inputs["bo"].astype(np.float32)
    out = np.empty((B, T, D), dtype=np.float32)
    for b in range(B):
        out[b] = res.results[2 * b]["out_p"] + res.results[2 * b + 1]["out_p"] + bo
    return res, out


def kernel(**inputs) -> np.ndarray:
    _, out = run_spmd(inputs, trace=False)
    return out
